# revision 27
# baseline (speedup 1.0000x reference)
"""Trainium2 Bass kernel for nn_MultiHeadAttention (B=2, S=4096, D=512, H=8).

Computes: q/k/v = relu(x@W+b) per head, softmax(q k^T / sqrt(64)) v,
out = relu(concat_heads @ Wo + bo).

Sharding: 8 cores = 2 (batch) x 4 (query-slice).  Each core computes full
K/V projections for its batch (redundant across the 4 q-slice cores) and
attention + output projection for its 1024-row query slice.  No collectives;
the host concatenates the 8 output slices.

Host-side prep (part of the sharding/layout step, not device compute):
x is cast to bf16 and transposed to feature-major x^T per batch, and the
weight matrices are cast to bf16 — the tensor engine contracts along the
partition dim, so all device matmuls consume feature-major operands.

Per-core kernel (all matmuls bf16 with fp32 PSUM accumulation):
  - K^T, Q^T computed feature-major: lhsT=W tile, rhs=x^T.  Bias+relu fused
    on DVE (bias is per-partition in this layout).
  - V computed in natural [s, d] layout (lhsT = x^T tile, rhs = Wv); bias via
    a K=1 ones-row matmul; relu on DVE; stored per head with a ones column
    appended (V_pad) so the attention U matmul also produces the softmax
    denominator row for free.
  - scores^T = K^T_h.T @ Q^T_h per (head, ktile): K=64 contraction; heads are
    processed in pairs at base partitions 0/64 so the two matmuls run
    concurrently in different PE row-groups.
  - exp on ACT (scale=1/8 fused), no max-subtraction (relu'd q/k make scores
    bounded: measured range [0, 6.6]).  ACT exp is the kernel's throughput
    floor (~1 elem/lane/cycle): exp ops span 2 ktiles x 2 heads (4 PSUM
    banks) to amortize the per-op overhead, the first attention block is
    interleaved with the K/V projection chunks, and the remaining
    projections are emitted between attention blocks so the PE does them
    inside ACT-bound stretches.
  - U^T[65, q] = V_pad_h.T @ P^T accumulated over ktiles in PSUM; row 64 is
    the denominator.  U^T is copied to SBUF immediately (releases the PSUM
    accumulator for the next block), then normalized off the critical path:
    DVE reciprocal + gpsimd partition broadcast + DVE multiply into
    feature-major O^T.
  - out = relu(O^T.T @ Wo + bo) via lhsT=O^T tiles, rhs=Wo; bias via ones-row
    matmul; relu on ACT; DMA to HBM.
"""

import numpy as np
import ml_dtypes

import concourse.bass as bass
import concourse.mybir as mybir
import concourse.tile as tile
from concourse import bacc
from concourse import bass_utils

F32 = mybir.dt.float32
BF16 = mybir.dt.bfloat16
AF = mybir.ActivationFunctionType
ALU = mybir.AluOpType

P = 128
D = 512
H = 8
DH = 64
DT = D // P  # 4 (also = number of head pairs)
B = 2
S = 4096
NCORES = 8
QSPLIT = 4
SQ_FULL = S // QSPLIT  # 1024 query rows per core
QC = 512               # q-chunk (matmul free dim / PSUM bank width)


def build_mha(sk=S, sq=SQ_FULL, skip_vbias=False):
    """Build the SPMD Bass program (identical on all cores).

    All inputs arrive pre-tiled by the host into exact SBUF layout
    ([128 partitions, contiguous free bytes]) so every load is a max-packet
    linear DMA."""
    nc = bacc.Bacc("TRN2", target_bir_lowering=False, debug=False,
                   num_devices=NCORES)

    xT_d = nc.dram_tensor("xT_bf", (P, DT * sk), BF16,
                          kind="ExternalInput").ap()  # chunk-major, see prep
    xqT_d = nc.dram_tensor("xqT_bf", (P, DT * sq), BF16,
                           kind="ExternalInput").ap()
    w_dram = {}
    for n in ("wq", "wk", "wv", "wo"):
        w_dram[n] = nc.dram_tensor(n, (P, DT * D), BF16,
                                   kind="ExternalInput").ap()
    b_dram = {
        "bq": nc.dram_tensor("bq", (P, DT), F32, kind="ExternalInput").ap(),
        "bk": nc.dram_tensor("bk", (P, DT), F32, kind="ExternalInput").ap(),
        "bv": nc.dram_tensor("bv", (1, D), BF16, kind="ExternalInput").ap(),
        "bo": nc.dram_tensor("bo", (1, D), BF16, kind="ExternalInput").ap(),
    }
    out = nc.dram_tensor("out", (sq, D), F32, kind="ExternalOutput").ap()

    with tile.TileContext(nc) as tc:
        _build_tile(tc, xT_d, xqT_d, w_dram, b_dram, out, sk, sq,
                    skip_vbias)

    nc.compile()
    return nc


def _build_tile(tc, xT_d, xqT_d, w_dram, b_dram, out, sk, sq,
                skip_vbias=False):
    nc = tc.nc
    SK_T = sk // P            # ktiles of the key/value sequence
    SQ_T = sq // P
    NQC = sq // QC            # q chunks per core
    CH = min(4, SK_T)         # stiles per projection chunk
    NCH = SK_T // CH
    KG = 1                    # ktiles per exp group

    with (
        tc.tile_pool(name="singles", bufs=1) as singles,
        tc.tile_pool(name="work", bufs=3) as work,
        tc.tile_pool(name="psum", bufs=2, space="PSUM") as psum,
    ):
        # ---- startup: only what Q-proj pair 0 needs, first ----
        w_bf = {}
        w_bf["wq"] = singles.tile([P, DT, D], BF16, name="wq_bf")
        nc.sync.dma_start(w_bf["wq"], w_dram["wq"].rearrange(
            "p (t n) -> p t n", t=DT))
        b_col = {}
        b_col["bq"] = singles.tile([P, DT], F32, name="bq_col")
        nc.sync.dma_start(b_col["bq"], b_dram["bq"])
        xTq = singles.tile([P, DT, sq], BF16)
        nc.sync.dma_start(xTq, xqT_d.rearrange("p (t s) -> p t s", t=DT))

        QT = singles.tile([P, DT, sq], BF16)

        def qproj(j, nq):
            psQ = psum.tile([P, QC], F32, tag="proj", name="psQ")
            for kt in range(DT):
                nc.tensor.matmul(
                    psQ, w_bf["wq"][:, kt, j * P:(j + 1) * P],
                    xTq[:, kt, nq * QC:(nq + 1) * QC],
                    start=(kt == 0), stop=(kt == DT - 1))
            nc.vector.tensor_scalar(
                QT[:, j, nq * QC:(nq + 1) * QC], psQ,
                b_col["bq"][:, j:j + 1], 0.0, op0=ALU.add, op1=ALU.max)

        qproj(0, 0)

        # ---- K-proj deps next (attention can start before V exists) ----
        b_row = {}
        w_bf["wk"] = singles.tile([P, DT, D], BF16, name="wk_bf")
        nc.sync.dma_start(w_bf["wk"], w_dram["wk"].rearrange(
            "p (t n) -> p t n", t=DT))
        b_col["bk"] = singles.tile([P, DT], F32, name="bk_col")
        nc.sync.dma_start(b_col["bk"], b_dram["bk"])
        CHP = CH * P
        xT = singles.tile([P, NCH, DT, CHP], BF16)
        xT_src = xT_d.rearrange("p (n t s) -> p n t s", n=NCH, t=DT)
        nc.sync.dma_start(xT[:, 0], xT_src[:, 0])
        for n in ("wv", "wo"):
            wb = singles.tile([P, DT, D], BF16, name=f"{n}_bf")
            nc.sync.dma_start(wb, w_dram[n].rearrange(
                "p (t n) -> p t n", t=DT))
            w_bf[n] = wb
            if n == "wv":
                br = singles.tile([1, D], BF16, name="bv_row")
                nc.sync.dma_start(br, b_dram["bv"])
                b_row["bv"] = br
        br = singles.tile([1, D], BF16, name="bo_row")
        nc.sync.dma_start(br, b_dram["bo"])
        b_row["bo"] = br

        # ---- persistent SBUF tensors ----
        xT1 = singles.tile([1, sk], BF16)
        nc.vector.memset(xT1, 1.0)
        KT = singles.tile([P, DT, sk], BF16)
        V_pad = singles.tile([P, SK_T, H, DH + 1], BF16)
        nc.vector.memset(V_pad[:, :, :, DH:DH + 1], 1.0)
        OT = singles.tile([P, DT, sq], BF16)
        OT1 = singles.tile([1, sq], BF16)
        nc.vector.memset(OT1, 1.0)

        # PSUM tags: "proj" 2x1 banks, "scores" 1x4 banks, "psU" 2x1 = 8
        def vproj(st):
            n, si = st // CH, st % CH
            psV = psum.tile([P, D], F32, tag="proj", name="psV")
            for kt in range(DT):
                nc.tensor.matmul(
                    psV, xT[:, n, kt, si * P:(si + 1) * P],
                    w_bf["wv"][:, kt, :],
                    start=(kt == 0),
                    stop=(skip_vbias and kt == DT - 1))
            if not skip_vbias:
                nc.tensor.matmul(psV, xT1[:, st * P:(st + 1) * P],
                                 b_row["bv"], start=False, stop=True)
            nc.vector.tensor_scalar_max(
                V_pad[:, st, :, 0:DH],
                psV.rearrange("p (h d) -> p h d", h=H), 0.0)

        def kproj(j, n):
            psK = psum.tile([P, CH * P], F32, tag="proj", name="psK")
            for kt in range(DT):
                nc.tensor.matmul(
                    psK, w_bf["wk"][:, kt, j * P:(j + 1) * P],
                    xT[:, n, kt, :],
                    start=(kt == 0), stop=(kt == DT - 1))
            nc.vector.tensor_scalar(
                KT[:, j, n * CH * P:(n + 1) * CH * P], psK,
                b_col["bk"][:, j:j + 1], 0.0, op0=ALU.add, op1=ALU.max)

        def attn_qk_exp(j, qc, kt0, nkt):
            """Scores + exp for nkt ktiles x 2 heads -> one ACT op."""
            q0 = qc * QC
            psS = psum.tile([P, 2 * KG * QC], F32, tag="scores", bufs=2,
                            name="psS")
            for i in range(nkt):
                kt = kt0 + i
                nc.tensor.matmul(
                    psS[:, (2 * i) * QC:(2 * i + 1) * QC],
                    KT[0:DH, j, kt * P:(kt + 1) * P],
                    QT[0:DH, j, q0:q0 + QC], start=True, stop=True)
                nc.tensor.matmul(
                    psS[:, (2 * i + 1) * QC:(2 * i + 2) * QC],
                    KT[DH:P, j, kt * P:(kt + 1) * P],
                    QT[DH:P, j, q0:q0 + QC], start=True, stop=True)
            pT = work.tile([P, 2 * KG * QC], BF16, tag="pT", bufs=5,
                           name="pT")
            nc.scalar.activation(pT[:, :2 * nkt * QC], psS[:, :2 * nkt * QC],
                                 AF.Exp, scale=0.125)
            return pT

        def attn_u(j, kt0, nkt, pT, psU_A, psU_B):
            for i in range(nkt):
                kt = kt0 + i
                first, last = (kt == 0), (kt == SK_T - 1)
                nc.tensor.matmul(psU_A, V_pad[:, kt, 2 * j, :],
                                 pT[:, (2 * i) * QC:(2 * i + 1) * QC],
                                 start=first, stop=last)
                nc.tensor.matmul(psU_B, V_pad[:, kt, 2 * j + 1, :],
                                 pT[:, (2 * i + 1) * QC:(2 * i + 2) * QC],
                                 start=first, stop=last)

        def attn_group(j, qc, kt0, nkt, psU_A, psU_B):
            pT = attn_qk_exp(j, qc, kt0, nkt)
            attn_u(j, kt0, nkt, pT, psU_A, psU_B)

        def attn_finish_copies(psU_A, psU_B):
            """Copy U out of PSUM fast — frees both accumulators for the
            next block.  Returns the SBUF copies."""
            ucs = []
            for psU in (psU_A, psU_B):
                uc = work.tile([DH + 1, QC], F32, tag="ucopy", bufs=6,
                               name="uc")
                nc.vector.tensor_copy(uc, psU)
                ucs.append(uc)
            return ucs

        brc_sink = {}

        def normalize_thunks(j, qc, ucs):
            """Per-head softmax normalize emitted later (as fillers inside
            the next block) so its latency hides under ACT-bound stretches."""
            q0 = qc * QC

            def one(uc, h0):
                def t():
                    recip = work.tile([1, QC], F32, tag="recip", bufs=4,
                                      name="recip")
                    nc.vector.reciprocal(recip, uc[DH:DH + 1, :])
                    brc = work.tile([DH, QC], F32, tag="brc", bufs=4,
                                    name="brc")
                    nc.gpsimd.partition_broadcast(brc, recip)
                    nc.vector.tensor_mul(
                        OT[h0:h0 + DH, j, q0:q0 + QC], uc[0:DH, :], brc)
                    brc_sink[(j, qc)] = brc
                return t
            return [one(ucs[0], 0), one(ucs[1], DH)]

        def attn_span(j, qc, kts, psU, fillers=()):
            """Emit the kt groups of one attention block, sprinkling `fillers`
            (deferred work thunks) between groups so the in-order PE/DVE do
            them inside this ACT-bound stretch.  Returns this block's
            normalize thunks (to be run as fillers of the NEXT block)."""
            fillers = list(fillers)
            ngroups = (len(kts) + KG - 1) // KG
            spacing = max(1, ngroups // (len(fillers) + 1))
            gi = 0
            for kt0 in range(kts[0], kts[0] + len(kts), KG):
                nkt = min(KG, kts[-1] + 1 - kt0)
                attn_group(j, qc, kt0, nkt, psU[0], psU[1])
                gi += 1
                if fillers and gi % spacing == 0:
                    fillers.pop(0)()
            for f in fillers:
                f()
            if kts[-1] == SK_T - 1:
                ucs = attn_finish_copies(psU[0], psU[1])
                return normalize_thunks(j, qc, ucs), ucs
            return [], None

        def new_psU():
            a = psum.tile([DH + 1, QC], F32, tag="psU", name="psU_A")
            b = psum.tile([DH + 1, QC], F32, tag="psU", name="psU_B")
            return (a, b)

        def outproj(qt):
            # bias matmul first: it reads OT1, whose re-write after the last
            # normalize acts as a scheduling gate for the whole chain (the
            # scheduler otherwise hoists these into mid-attention PE-idle
            # slots and stalls on under-modeled reciprocal latency)
            psO = psum.tile([P, D], F32, tag="proj", name="psO")
            nc.tensor.matmul(psO, OT1[:, qt * P:(qt + 1) * P],
                             b_row["bo"], start=True, stop=False)
            for j in range(DT):
                nc.tensor.matmul(psO, OT[:, j, qt * P:(qt + 1) * P],
                                 w_bf["wo"][:, j, :],
                                 start=False, stop=(j == DT - 1))
            o_sb = work.tile([P, D], F32, tag="osb", bufs=2, name="o_sb")
            nc.scalar.activation(o_sb, psO, AF.Relu)
            nc.sync.dma_start(out[qt * P:(qt + 1) * P, :], o_sb)

        def gate_outproj(blk):
            """No-op rewrite of OT1 (max(1, recip<1) == 1) that depends on
            block `blk`'s normalize chain — gates the outproj chains (which
            start with an OT1-reading bias matmul) behind it, preventing the
            scheduler from hoisting them into mid-attention stalls."""
            brc = brc_sink[blk]
            nc.vector.tensor_scalar(OT1, OT1, brc[0:1, 0:1], None,
                                    op0=ALU.max)

        # ---- chunk loop: x load + V proj + K proj(pair 0) + attn(0, 0) ----
        psU0 = new_psU()
        pendq = []   # queue of deferred normalize-thunk lists (2-block lag)
        for n in range(NCH):
            if n > 0:
                nc.sync.dma_start(xT[:, n], xT_src[:, n])
            kproj(0, n)
            kts = list(range(n * CH, (n + 1) * CH))
            # QK + exp first: ACT can start before V exists (only U needs V)
            pTs = [(kt, attn_qk_exp(0, 0, kt, 1)) for kt in kts]
            for st in kts:
                vproj(st)
            for kt, pT in pTs:
                attn_u(0, kt, 1, pT, psU0[0], psU0[1])
            if kts[-1] == SK_T - 1:
                ucs0 = attn_finish_copies(psU0[0], psU0[1])
                thunks = normalize_thunks(0, 0, ucs0)
        pendq.append(thunks)

        # ---- remaining attention; fillers inside each ACT-bound block are:
        # the previous block's normalize chain + the next block's
        # projections (+ the qc0 half of the output projection during the
        # last block) ----
        blocks = [(0, qc) for qc in range(1, NQC)]
        blocks += [(j, qc) for j in range(1, DT) for qc in range(NQC)]
        owed = {blk: [] for blk in blocks}
        for (j, qc) in blocks:
            owed[(j, qc)].append(lambda j=j, qc=qc: qproj(j, qc))
            if qc == 0 and j >= 1:
                for n in range(NCH):
                    owed[(j, qc)].append(lambda j=j, n=n: kproj(j, n))
        for f in owed[blocks[0]]:
            f()
        for bi, (j, qc) in enumerate(blocks):
            # projection fillers first; normalize chains run with a 2-block
            # lag so their slow DVE reciprocals never sit near a block
            # boundary (where they would delay the relus feeding the next
            # pair's attention)
            fillers = []
            if bi + 1 < len(blocks):
                fillers += owed[blocks[bi + 1]]
            last = bi == len(blocks) - 1
            if last:
                # flush remaining normalize chains, then gate + emit the qc0
                # half of the output projection so it runs inside this block
                while pendq:
                    fillers += pendq.pop(0)
                if NQC > 1:
                    fillers += [lambda: gate_outproj((DT - 1, 0))]
                    fillers += [lambda qt=qt: outproj(qt)
                                for qt in range(SQ_T // NQC)]
            elif len(pendq) >= 2:
                fillers += pendq.pop(0)
                if bi == len(blocks) - 2 and pendq:
                    fillers += pendq.pop(0)
            psU = new_psU()
            thunks, ucs = attn_span(j, qc, list(range(SK_T)), psU, fillers)
            pendq.append(thunks)

        # ---- tail: last block's normalize + remaining output rows ----
        while pendq:
            for f in pendq.pop(0):
                f()
        gate_outproj(blocks[-1])
        for qt in range(SQ_T // NQC if NQC > 1 else 0, SQ_T):
            outproj(qt)


_NC_CACHE = {}


def _get_nc(sk=S, sq=SQ_FULL, skip_vbias=False):
    key = (sk, sq, skip_vbias)
    if key not in _NC_CACHE:
        _NC_CACHE[key] = build_mha(sk, sq, skip_vbias)
    return _NC_CACHE[key]


def _tile_rows(a):
    """[D, n] -> SBUF layout [P, DT*n]: partition p gets rows p, 128+p, ..."""
    Dd, n = a.shape
    t = Dd // P
    return np.ascontiguousarray(
        a.reshape(t, P, n).transpose(1, 0, 2).reshape(P, t * n))


def _tile_chunks(a, chp):
    """[D, sk] -> chunk-major SBUF layout [P, NCH*DT*chp]: per partition,
    sequence chunks outermost so each chunk is one contiguous linear DMA."""
    Dd, sk = a.shape
    t, nch = Dd // P, sk // chp
    return np.ascontiguousarray(
        a.reshape(t, P, nch, chp).transpose(1, 2, 0, 3).reshape(P, -1))


def prep_inputs(x, Wq, bq, Wk, bk, Wv, bv, Wo, bo):
    """Host-side sharding/layout prep: bf16 casts, feature-major transpose,
    SBUF pre-tiling.  Returns the 8 per-core input maps."""
    bf = ml_dtypes.bfloat16
    x = np.asarray(x, dtype=np.float32)
    shared = {
        "wq": _tile_rows(np.asarray(Wq, np.float32).astype(bf)),
        "wk": _tile_rows(np.asarray(Wk, np.float32).astype(bf)),
        "wv": _tile_rows(np.asarray(Wv, np.float32).astype(bf)),
        "wo": _tile_rows(np.asarray(Wo, np.float32).astype(bf)),
        "bq": np.ascontiguousarray(
            np.asarray(bq, np.float32).reshape(DT, P).T),
        "bk": np.ascontiguousarray(
            np.asarray(bk, np.float32).reshape(DT, P).T),
        "bv": np.asarray(bv, np.float32).astype(bf).reshape(1, D),
        "bo": np.asarray(bo, np.float32).astype(bf).reshape(1, D),
    }
    xT_b = [x[b].T.astype(bf) for b in range(B)]
    xT_tiled = [_tile_chunks(xb, 4 * P) for xb in xT_b]
    in_maps = []
    for c in range(NCORES):
        b, qo = divmod(c, QSPLIT)
        m = dict(shared)
        m["xT_bf"] = xT_tiled[b]
        m["xqT_bf"] = _tile_rows(
            xT_b[b][:, qo * SQ_FULL:(qo + 1) * SQ_FULL])
        in_maps.append(m)
    return in_maps


def kernel(x, Wq, bq, Wk, bk, Wv, bv, Wo, bo, **run_kwargs):
    """Full-input entry point: shards across 8 NeuronCores, returns full out."""
    in_maps = prep_inputs(x, Wq, bq, Wk, bk, Wv, bv, Wo, bo)
    nc = _get_nc(skip_vbias=bool(np.all(np.asarray(bv) == 0)))
    res = bass_utils.run_bass_kernel_spmd(
        nc, in_maps, core_ids=list(range(NCORES)), **run_kwargs)
    full = np.empty((B, S, D), np.float32)
    for c in range(NCORES):
        b, qo = divmod(c, QSPLIT)
        full[b, qo * SQ_FULL:(qo + 1) * SQ_FULL] = res.results[c]["out"]
    if run_kwargs:
        return full, res
    return full


# revision 34
# speedup vs baseline: 1.1932x; 1.1932x over previous
"""Trainium2 Bass kernel for nn_MultiHeadAttention (B=2, S=4096, D=512, H=8).

Computes: q/k/v = relu(x@W+b) per head, softmax(q k^T / sqrt(64)) v,
out = relu(concat_heads @ Wo + bo).

Sharding: 8 cores = 2 (batch) x 4 (query-slice).  Each core computes full
K/V projections for its batch (redundant across the 4 q-slice cores) and
attention + output projection for its 1024-row query slice.  No collectives;
the host concatenates the 8 output slices.

Host-side prep (part of the sharding/layout step, not device compute):
x is cast to bf16 and transposed to feature-major x^T per batch, and the
weight matrices are cast to bf16 — the tensor engine contracts along the
partition dim, so all device matmuls consume feature-major operands.

Per-core kernel (all matmuls bf16 with fp32 PSUM accumulation):
  - K^T, Q^T computed feature-major: lhsT=W tile, rhs=x^T.  Bias+relu fused
    on DVE (bias is per-partition in this layout).
  - V computed in natural [s, d] layout (lhsT = x^T tile, rhs = Wv); bias via
    a K=1 ones-row matmul; relu on DVE; stored per head with a ones column
    appended (V_pad) so the attention U matmul also produces the softmax
    denominator row for free.
  - scores^T = K^T_h.T @ Q^T_h per (head, ktile): K=64 contraction; heads are
    processed in pairs at base partitions 0/64 so the two matmuls run
    concurrently in different PE row-groups.
  - exp on ACT (scale=1/8 fused), no max-subtraction (relu'd q/k make scores
    bounded: measured range [0, 6.6]).  ACT exp is the kernel's throughput
    floor (~1 elem/lane/cycle): exp ops span 2 ktiles x 2 heads (4 PSUM
    banks) to amortize the per-op overhead, the first attention block is
    interleaved with the K/V projection chunks, and the remaining
    projections are emitted between attention blocks so the PE does them
    inside ACT-bound stretches.
  - U^T[65, q] = V_pad_h.T @ P^T accumulated over ktiles in PSUM; row 64 is
    the denominator.  U^T is copied to SBUF immediately (releases the PSUM
    accumulator for the next block), then normalized off the critical path:
    DVE reciprocal + gpsimd partition broadcast + DVE multiply into
    feature-major O^T.
  - out = relu(O^T.T @ Wo + bo) via lhsT=O^T tiles, rhs=Wo; bias via ones-row
    matmul; relu on ACT; DMA to HBM.
"""

import numpy as np
import ml_dtypes

import concourse.bass as bass
import concourse.mybir as mybir
import concourse.tile as tile
from concourse import bacc
from concourse import bass_utils

F32 = mybir.dt.float32
BF16 = mybir.dt.bfloat16
AF = mybir.ActivationFunctionType
ALU = mybir.AluOpType

P = 128
D = 512
H = 8
DH = 64
DT = D // P  # 4 (also = number of head pairs)
B = 2
S = 4096
NCORES = 8
QSPLIT = 4
SQ_FULL = S // QSPLIT  # 1024 query rows per core
QC = 512               # q-chunk (matmul free dim / PSUM bank width)


def build_mha(sk=S, sq=SQ_FULL, skip_vbias=False):
    """Build the SPMD Bass program (identical on all cores).

    All inputs arrive pre-tiled by the host into exact SBUF layout
    ([128 partitions, contiguous free bytes]) so every load is a max-packet
    linear DMA."""
    nc = bacc.Bacc("TRN2", target_bir_lowering=False, debug=False,
                   num_devices=NCORES)

    xT_d = nc.dram_tensor("xT_bf", (P, DT * sk), BF16,
                          kind="ExternalInput").ap()  # chunk-major, see prep
    xqT_d = nc.dram_tensor("xqT_bf", (P, DT * sq), BF16,
                           kind="ExternalInput").ap()
    w_dram = {}
    for n in ("wq", "wk", "wv", "wo"):
        w_dram[n] = nc.dram_tensor(n, (P, DT * D), BF16,
                                   kind="ExternalInput").ap()
    b_dram = {
        "bq": nc.dram_tensor("bq", (P, DT), F32, kind="ExternalInput").ap(),
        "bk": nc.dram_tensor("bk", (P, DT), F32, kind="ExternalInput").ap(),
        "bv": nc.dram_tensor("bv", (1, D), BF16, kind="ExternalInput").ap(),
        "bo": nc.dram_tensor("bo", (1, D), BF16, kind="ExternalInput").ap(),
    }
    out = nc.dram_tensor("out", (sq, D), F32, kind="ExternalOutput").ap()

    with tile.TileContext(nc) as tc:
        _build_tile(tc, xT_d, xqT_d, w_dram, b_dram, out, sk, sq,
                    skip_vbias)

    nc.compile()
    return nc


def _build_tile(tc, xT_d, xqT_d, w_dram, b_dram, out, sk, sq,
                skip_vbias=False):
    nc = tc.nc
    SK_T = sk // P            # ktiles of the key/value sequence
    SQ_T = sq // P
    NQC = sq // QC            # q chunks per core
    CH = min(4, SK_T)         # stiles per projection chunk
    NCH = SK_T // CH
    KG = 1                    # ktiles per exp group

    with (
        tc.tile_pool(name="singles", bufs=1) as singles,
        tc.tile_pool(name="work", bufs=3) as work,
        tc.tile_pool(name="psum", bufs=2, space="PSUM") as psum,
    ):
        # ---- startup: only what Q-proj pair 0 needs, first ----
        w_bf = {}
        w_bf["wq"] = singles.tile([P, DT, D], BF16, name="wq_bf")
        nc.sync.dma_start(w_bf["wq"], w_dram["wq"].rearrange(
            "p (t n) -> p t n", t=DT))
        b_col = {}
        b_col["bq"] = singles.tile([P, DT], F32, name="bq_col")
        nc.sync.dma_start(b_col["bq"], b_dram["bq"])
        xTq = singles.tile([P, DT, sq], BF16)
        nc.sync.dma_start(xTq, xqT_d.rearrange("p (t s) -> p t s", t=DT))

        QT = singles.tile([P, DT, sq], BF16)

        def qproj(j, nq):
            psQ = psum.tile([P, QC], F32, tag="proj", name="psQ")
            for kt in range(DT):
                nc.tensor.matmul(
                    psQ, w_bf["wq"][:, kt, j * P:(j + 1) * P],
                    xTq[:, kt, nq * QC:(nq + 1) * QC],
                    start=(kt == 0), stop=(kt == DT - 1))
            nc.vector.tensor_scalar(
                QT[:, j, nq * QC:(nq + 1) * QC], psQ,
                b_col["bq"][:, j:j + 1], 0.0, op0=ALU.add, op1=ALU.max)

        qproj(0, 0)
        if NQC > 1:
            qproj(0, 1)

        # ---- K-proj deps next (attention can start before V exists) ----
        b_row = {}
        w_bf["wk"] = singles.tile([P, DT, D], BF16, name="wk_bf")
        nc.sync.dma_start(w_bf["wk"], w_dram["wk"].rearrange(
            "p (t n) -> p t n", t=DT))
        b_col["bk"] = singles.tile([P, DT], F32, name="bk_col")
        nc.sync.dma_start(b_col["bk"], b_dram["bk"])
        CHP = CH * P
        xT = singles.tile([P, NCH, DT, CHP], BF16)
        xT_src = xT_d.rearrange("p (n t s) -> p n t s", n=NCH, t=DT)
        nc.sync.dma_start(xT[:, 0], xT_src[:, 0])
        for n in ("wv", "wo"):
            wb = singles.tile([P, DT, D], BF16, name=f"{n}_bf")
            nc.sync.dma_start(wb, w_dram[n].rearrange(
                "p (t n) -> p t n", t=DT))
            w_bf[n] = wb
            if n == "wv":
                br = singles.tile([1, D], BF16, name="bv_row")
                nc.sync.dma_start(br, b_dram["bv"])
                b_row["bv"] = br
        br = singles.tile([1, D], BF16, name="bo_row")
        nc.sync.dma_start(br, b_dram["bo"])
        b_row["bo"] = br

        # ---- persistent SBUF tensors ----
        xT1 = singles.tile([1, sk], BF16)
        nc.vector.memset(xT1, 1.0)
        KT = singles.tile([P, DT, sk], BF16)
        V_pad = singles.tile([P, SK_T, H, DH + 1], BF16)
        nc.vector.memset(V_pad[:, :, :, DH:DH + 1], 1.0)
        OT = singles.tile([P, DT, sq], BF16)
        OT1 = singles.tile([1, sq], BF16)
        nc.vector.memset(OT1, 1.0)

        # PSUM tags: "proj" 2x1 banks, "scores" 1x4 banks, "psU" 2x1 = 8
        def vproj(st):
            n, si = st // CH, st % CH
            psV = psum.tile([P, D], F32, tag="proj", name="psV")
            for kt in range(DT):
                nc.tensor.matmul(
                    psV, xT[:, n, kt, si * P:(si + 1) * P],
                    w_bf["wv"][:, kt, :],
                    start=(kt == 0),
                    stop=(skip_vbias and kt == DT - 1))
            if not skip_vbias:
                nc.tensor.matmul(psV, xT1[:, st * P:(st + 1) * P],
                                 b_row["bv"], start=False, stop=True)
            nc.vector.tensor_scalar_max(
                V_pad[:, st, :, 0:DH],
                psV.rearrange("p (h d) -> p h d", h=H), 0.0)

        def kproj(j, n):
            psK = psum.tile([P, CH * P], F32, tag="proj", name="psK")
            for kt in range(DT):
                nc.tensor.matmul(
                    psK, w_bf["wk"][:, kt, j * P:(j + 1) * P],
                    xT[:, n, kt, :],
                    start=(kt == 0), stop=(kt == DT - 1))
            nc.vector.tensor_scalar(
                KT[:, j, n * CH * P:(n + 1) * CH * P], psK,
                b_col["bk"][:, j:j + 1], 0.0, op0=ALU.add, op1=ALU.max)

        def attn_qk_exp(j, qc, kt0, nkt, pt_tag="pT", pt_bufs=5):
            """Scores + exp for nkt ktiles x 2 heads -> one ACT op."""
            q0 = qc * QC
            psS = psum.tile([P, 2 * KG * QC], F32, tag="scores", bufs=2,
                            name="psS")
            for i in range(nkt):
                kt = kt0 + i
                nc.tensor.matmul(
                    psS[:, (2 * i) * QC:(2 * i + 1) * QC],
                    KT[0:DH, j, kt * P:(kt + 1) * P],
                    QT[0:DH, j, q0:q0 + QC], start=True, stop=True)
                nc.tensor.matmul(
                    psS[:, (2 * i + 1) * QC:(2 * i + 2) * QC],
                    KT[DH:P, j, kt * P:(kt + 1) * P],
                    QT[DH:P, j, q0:q0 + QC], start=True, stop=True)
            pT = work.tile([P, 2 * KG * QC], BF16, tag=pt_tag,
                           bufs=pt_bufs, name="pT")
            nc.scalar.activation(pT[:, :2 * nkt * QC], psS[:, :2 * nkt * QC],
                                 AF.Exp, scale=0.125)
            return pT

        def attn_u(j, kt0, nkt, pT, psU_A, psU_B):
            for i in range(nkt):
                kt = kt0 + i
                first, last = (kt == 0), (kt == SK_T - 1)
                nc.tensor.matmul(psU_A, V_pad[:, kt, 2 * j, :],
                                 pT[:, (2 * i) * QC:(2 * i + 1) * QC],
                                 start=first, stop=last)
                nc.tensor.matmul(psU_B, V_pad[:, kt, 2 * j + 1, :],
                                 pT[:, (2 * i + 1) * QC:(2 * i + 2) * QC],
                                 start=first, stop=last)

        def attn_group(j, qc, kt0, nkt, psU_A, psU_B):
            pT = attn_qk_exp(j, qc, kt0, nkt)
            attn_u(j, kt0, nkt, pT, psU_A, psU_B)

        def attn_finish_copies(psU_A, psU_B):
            """Copy U out of PSUM fast — frees both accumulators for the
            next block.  Returns the SBUF copies."""
            ucs = []
            for psU in (psU_A, psU_B):
                uc = work.tile([DH + 1, QC], F32, tag="ucopy", bufs=6,
                               name="uc")
                nc.vector.tensor_copy(uc, psU)
                ucs.append(uc)
            return ucs

        brc_sink = {}

        def normalize_thunks(j, qc, ucs):
            """Per-head softmax normalize emitted later (as fillers inside
            the next block) so its latency hides under ACT-bound stretches."""
            q0 = qc * QC

            def one(uc, h0):
                def t():
                    recip = work.tile([1, QC], F32, tag="recip", bufs=4,
                                      name="recip")
                    nc.vector.reciprocal(recip, uc[DH:DH + 1, :])
                    brc = work.tile([DH, QC], F32, tag="brc", bufs=4,
                                    name="brc")
                    nc.gpsimd.partition_broadcast(brc, recip)
                    nc.vector.tensor_mul(
                        OT[h0:h0 + DH, j, q0:q0 + QC], uc[0:DH, :], brc)
                    brc_sink[(j, qc)] = brc
                return t
            return [one(ucs[0], 0), one(ucs[1], DH)]

        def attn_span(j, qc, kts, psU, fillers=(), precomputed=()):
            """Emit the kt groups of one attention block, sprinkling `fillers`
            (deferred work thunks) between groups so the in-order PE/DVE do
            them inside this ACT-bound stretch.  Returns this block's
            normalize thunks (to be run as fillers of the NEXT block)."""
            fillers = list(fillers)
            for kt, pT in precomputed:
                attn_u(j, kt, 1, pT, psU[0], psU[1])
            ngroups = (len(kts) + KG - 1) // KG
            spacing = max(1, ngroups // (len(fillers) + 1))
            gi = 0
            for kt0 in range(kts[0], kts[0] + len(kts), KG):
                nkt = min(KG, kts[-1] + 1 - kt0)
                attn_group(j, qc, kt0, nkt, psU[0], psU[1])
                gi += 1
                if fillers and gi % spacing == 0:
                    fillers.pop(0)()
            for f in fillers:
                f()
            if kts[-1] == SK_T - 1:
                ucs = attn_finish_copies(psU[0], psU[1])
                return normalize_thunks(j, qc, ucs), ucs
            return [], None

        def new_psU():
            a = psum.tile([DH + 1, QC], F32, tag="psU", name="psU_A")
            b = psum.tile([DH + 1, QC], F32, tag="psU", name="psU_B")
            return (a, b)

        def outproj(qt):
            # bias matmul first: it reads OT1, whose re-write after the last
            # normalize acts as a scheduling gate for the whole chain (the
            # scheduler otherwise hoists these into mid-attention PE-idle
            # slots and stalls on under-modeled reciprocal latency)
            psO = psum.tile([P, D], F32, tag="proj", name="psO")
            nc.tensor.matmul(psO, OT1[:, qt * P:(qt + 1) * P],
                             b_row["bo"], start=True, stop=False)
            for j in range(DT):
                nc.tensor.matmul(psO, OT[:, j, qt * P:(qt + 1) * P],
                                 w_bf["wo"][:, j, :],
                                 start=False, stop=(j == DT - 1))
            o_sb = work.tile([P, D], F32, tag="osb", bufs=2, name="o_sb")
            nc.scalar.activation(o_sb, psO, AF.Relu)
            nc.sync.dma_start(out[qt * P:(qt + 1) * P, :], o_sb)

        def gate_outproj(blk):
            """No-op rewrite of OT1 (max(1, recip<1) == 1) that depends on
            block `blk`'s normalize chain — gates the outproj chains (which
            start with an OT1-reading bias matmul) behind it, preventing the
            scheduler from hoisting them into mid-attention stalls."""
            brc = brc_sink[blk]
            nc.vector.tensor_scalar(OT1, OT1, brc[0:1, 0:1], None,
                                    op0=ALU.max)

        # ---- chunk loop: x load + V proj + K proj(pair 0) + attn(0, 0) ----
        psU0 = new_psU()
        N_STORE = 8
        store01 = []
        pendq = []   # queue of deferred normalize-thunk lists (2-block lag)
        for n in range(NCH):
            if n > 0:
                nc.sync.dma_start(xT[:, n], xT_src[:, n])
            kproj(0, n)
            kts = list(range(n * CH, (n + 1) * CH))
            # QK + exp first: ACT can start before V exists (only U needs V)
            pTs = [(kt, attn_qk_exp(0, 0, kt, 1)) for kt in kts]
            for st in kts:
                vproj(st)
            for kt, pT in pTs:
                attn_u(0, kt, 1, pT, psU0[0], psU0[1])
            if NQC > 1 and n < N_STORE:
                # pre-compute one exp of block (0,1) per chunk into a held
                # pT: fills the otherwise-idle ACT during the PE-bound
                # chunk phase (the U matmuls run later, so no PSUM cost)
                store01.append((n, attn_qk_exp(0, 1, n, 1, pt_tag="pT01",
                                               pt_bufs=N_STORE)))
            if kts[-1] == SK_T - 1:
                ucs0 = attn_finish_copies(psU0[0], psU0[1])
                thunks = normalize_thunks(0, 0, ucs0)
        pendq.append(thunks)

        # ---- remaining attention; fillers inside each ACT-bound block are:
        # the previous block's normalize chain + the next block's
        # projections (+ the qc0 half of the output projection during the
        # last block) ----
        blocks = [(0, qc) for qc in range(1, NQC)]
        blocks += [(j, qc) for j in range(1, DT) for qc in range(NQC)]
        owed = {blk: [] for blk in blocks}
        for (j, qc) in blocks:
            if (j, qc) != (0, 1):
                owed[(j, qc)].append(lambda j=j, qc=qc: qproj(j, qc))
            if qc == 0 and j >= 1:
                for n in range(NCH):
                    owed[(j, qc)].append(lambda j=j, n=n: kproj(j, n))
        for f in owed[blocks[0]]:
            f()
        for bi, (j, qc) in enumerate(blocks):
            # projection fillers first; normalize chains run with a 2-block
            # lag so their slow DVE reciprocals never sit near a block
            # boundary (where they would delay the relus feeding the next
            # pair's attention)
            fillers = []
            if bi + 1 < len(blocks):
                fillers += owed[blocks[bi + 1]]
            last = bi == len(blocks) - 1
            if last:
                # flush remaining normalize chains, then gate + emit the qc0
                # half of the output projection so it runs inside this block
                while pendq:
                    fillers += pendq.pop(0)
                if NQC > 1:
                    fillers += [lambda: gate_outproj((DT - 1, 0))]
                    fillers += [lambda qt=qt: outproj(qt)
                                for qt in range(SQ_T // NQC)]
            elif len(pendq) >= 2:
                fillers += pendq.pop(0)
                if bi == len(blocks) - 2 and pendq:
                    fillers += pendq.pop(0)
            psU = new_psU()
            if (j, qc) == (0, 1) and store01:
                thunks, ucs = attn_span(
                    j, qc, list(range(len(store01), SK_T)), psU, fillers,
                    precomputed=store01)
            else:
                thunks, ucs = attn_span(j, qc, list(range(SK_T)), psU,
                                        fillers)
            pendq.append(thunks)
            last_ucs = ucs

        # ---- tail: last block's normalize + remaining output rows ----
        # Two of the final outproj chains are gated only on the last block's
        # PSUM copies (their bias + pairs-0..2 matmuls need nothing newer),
        # so the PE does useful work during the slow reciprocal chain and
        # stays HAM-warm; their pair-3 matmul still waits on the real OT
        # write.  Gate writes go on DVE BEFORE the normalize thunks so they
        # are not queued behind the reciprocals.
        qt_lo = SQ_T // NQC if NQC > 1 else 0
        early = []
        open_psO = []
        if NQC > 1 and last_ucs is not None:
            early = [qt_lo, qt_lo + 1]
            for qt, uc in zip(early, last_ucs):
                nc.vector.tensor_scalar(
                    OT1[:, qt * P:(qt + 1) * P],
                    OT1[:, qt * P:(qt + 1) * P],
                    uc[DH:DH + 1, 0:1], None, op0=ALU.min)
            # partial chains (bias + pairs 0..2): no pair-3 matmul yet, so
            # the in-order PE runs all 8 matmuls during the reciprocals
            # instead of stalling at the first chain's pair-3 wait
            for qt in early:
                psO = psum.tile([P, D], F32, tag="proj", name="psO")
                nc.tensor.matmul(psO, OT1[:, qt * P:(qt + 1) * P],
                                 b_row["bo"], start=True, stop=False)
                for j in range(DT - 1):
                    nc.tensor.matmul(psO, OT[:, j, qt * P:(qt + 1) * P],
                                     w_bf["wo"][:, j, :],
                                     start=False, stop=False)
                open_psO.append((qt, psO))
        while pendq:
            for f in pendq.pop(0):
                f()
        for qt, psO in open_psO:
            nc.tensor.matmul(psO, OT[:, DT - 1, qt * P:(qt + 1) * P],
                             w_bf["wo"][:, DT - 1, :],
                             start=False, stop=True)
            o_sb = work.tile([P, D], F32, tag="osb", bufs=2, name="o_sb")
            nc.scalar.activation(o_sb, psO, AF.Relu)
            nc.sync.dma_start(out[qt * P:(qt + 1) * P, :], o_sb)
        gate_outproj(blocks[-1])
        for qt in range(qt_lo, SQ_T):
            if qt not in early:
                outproj(qt)


_NC_CACHE = {}


def _get_nc(sk=S, sq=SQ_FULL, skip_vbias=False):
    key = (sk, sq, skip_vbias)
    if key not in _NC_CACHE:
        _NC_CACHE[key] = build_mha(sk, sq, skip_vbias)
    return _NC_CACHE[key]


def _tile_rows(a):
    """[D, n] -> SBUF layout [P, DT*n]: partition p gets rows p, 128+p, ..."""
    Dd, n = a.shape
    t = Dd // P
    return np.ascontiguousarray(
        a.reshape(t, P, n).transpose(1, 0, 2).reshape(P, t * n))


def _tile_chunks(a, chp):
    """[D, sk] -> chunk-major SBUF layout [P, NCH*DT*chp]: per partition,
    sequence chunks outermost so each chunk is one contiguous linear DMA."""
    Dd, sk = a.shape
    t, nch = Dd // P, sk // chp
    return np.ascontiguousarray(
        a.reshape(t, P, nch, chp).transpose(1, 2, 0, 3).reshape(P, -1))


def prep_inputs(x, Wq, bq, Wk, bk, Wv, bv, Wo, bo):
    """Host-side sharding/layout prep: bf16 casts, feature-major transpose,
    SBUF pre-tiling.  Returns the 8 per-core input maps."""
    bf = ml_dtypes.bfloat16
    x = np.asarray(x, dtype=np.float32)
    shared = {
        "wq": _tile_rows(np.asarray(Wq, np.float32).astype(bf)),
        "wk": _tile_rows(np.asarray(Wk, np.float32).astype(bf)),
        "wv": _tile_rows(np.asarray(Wv, np.float32).astype(bf)),
        "wo": _tile_rows(np.asarray(Wo, np.float32).astype(bf)),
        "bq": np.ascontiguousarray(
            np.asarray(bq, np.float32).reshape(DT, P).T),
        "bk": np.ascontiguousarray(
            np.asarray(bk, np.float32).reshape(DT, P).T),
        "bv": np.asarray(bv, np.float32).astype(bf).reshape(1, D),
        "bo": np.asarray(bo, np.float32).astype(bf).reshape(1, D),
    }
    xT_b = [x[b].T.astype(bf) for b in range(B)]
    xT_tiled = [_tile_chunks(xb, 4 * P) for xb in xT_b]
    in_maps = []
    for c in range(NCORES):
        b, qo = divmod(c, QSPLIT)
        m = dict(shared)
        m["xT_bf"] = xT_tiled[b]
        m["xqT_bf"] = _tile_rows(
            xT_b[b][:, qo * SQ_FULL:(qo + 1) * SQ_FULL])
        in_maps.append(m)
    return in_maps


def kernel(x, Wq, bq, Wk, bk, Wv, bv, Wo, bo, **run_kwargs):
    """Full-input entry point: shards across 8 NeuronCores, returns full out."""
    in_maps = prep_inputs(x, Wq, bq, Wk, bk, Wv, bv, Wo, bo)
    nc = _get_nc(skip_vbias=bool(np.all(np.asarray(bv) == 0)))
    res = bass_utils.run_bass_kernel_spmd(
        nc, in_maps, core_ids=list(range(NCORES)), **run_kwargs)
    full = np.empty((B, S, D), np.float32)
    for c in range(NCORES):
        b, qo = divmod(c, QSPLIT)
        full[b, qo * SQ_FULL:(qo + 1) * SQ_FULL] = res.results[c]["out"]
    if run_kwargs:
        return full, res
    return full
